# revision 1
# baseline (speedup 1.0000x reference)
"""Causal single-head attention, data-parallel across 8 TRN2 NeuronCores.

Problem: x [512, 128, 512] f32, Wq/Wk/Wv [64, 512] f32.
  Q = x @ Wq.T; K = x @ Wk.T; V = x @ Wv.T     (per batch, [T=128, H=64])
  out = softmax(causal(Q K^T / 8)) @ V          ([T, H])

Sharding: batch dim (512) split across 8 cores, 64 batches/core, no
collectives.  Host prep (layout only): x is passed transposed per batch
([B, C, T]) so the contraction dim lands on SBUF partitions without any
on-device transpose of the big tensor; weights are pre-transposed to
[C, H] (Wq pre-scaled by 1/8) and cast to bf16.

Per-core kernel (batches processed in pairs, bf16 compute, f32 I/O):
  - SWDGE cast-DMA loads xT f32 -> SBUF bf16 [c_local, pair, chunk, t]
  - PE: QT/KT/VT = W.T @ xT accumulated over 4 C-chunks; the two batches
    of a pair are column-tiled into one PSUM bank (b0 -> partitions
    0-63, b1 -> 64-127 via tile_position=(0, 64))
  - ACT copies K/V PSUM->SBUF bf16 in one op; Q is copied into
    persistent zero-padded [128, 2, T] tiles so the scores matmuls
    contract over the full 128 partitions with base-0 operands
    (partition-offset matmul/transpose operands crash this silicon)
  - ACT exp(S) PSUM->SBUF bf16; GPSIMD affine_select applies the causal
    mask (fill 0 where s > t)
  - V^T and P are transposed on the PE against a resident identity
  - the 32 batch-pairs run through an explicit 3-stage software
    pipeline (proj | scores | finish) so the PE queue never head-blocks
    on a previous pair's ACT/DVE work
  - V gets a ones-column appended so the PV matmul also produces the
    softmax row-sums (column 64); DVE reciprocal + broadcast multiply
    normalizes; HWDGE stores f32 out.
"""

import numpy as np
import ml_dtypes

import concourse.bass as bass
import concourse.mybir as mybir
import concourse.tile as tile
from concourse import bacc
from concourse.bass_utils import run_bass_kernel_spmd
from concourse.masks import make_identity

B, T, C, H = 512, 128, 512, 64
NCORES = 8
BPC = B // NCORES          # 64 batches per core
NPAIR = BPC // 2           # 32 pairs
NGRP = 8                   # load groups (4 pairs = 8 batches each)
GP = NPAIR // NGRP         # pairs per group
KCH = C // 128             # 4 contraction chunks

BF16 = mybir.dt.bfloat16
F32 = mybir.dt.float32

_cache = {}
TAIL2 = __import__("os").environ.get("TAIL2", "0") == "1"


def _build():
    nc = bacc.Bacc(
        "TRN2", target_bir_lowering=False, debug=False, enable_asserts=False
    )
    x_d = nc.dram_tensor(
        "x", [NGRP, 128, GP, 2, KCH, T], F32, kind="ExternalInput"
    ).ap()
    w_d = nc.dram_tensor("w", [C, 3, H], BF16, kind="ExternalInput").ap()
    out_d = nc.dram_tensor("out", [BPC, T, H], F32, kind="ExternalOutput").ap()

    with tile.TileContext(nc) as tc:
        with (
            tc.tile_pool(name="wpool", bufs=1) as wpool,
            tc.tile_pool(name="xtpool", bufs=2) as xtpool,
            tc.tile_pool(name="qkvpool", bufs=4) as qkvpool,
            tc.tile_pool(name="ppool", bufs=4) as ppool,
            tc.tile_pool(name="ptpool", bufs=4) as ptpool,
            tc.tile_pool(name="vpool", bufs=4) as vpool,
            tc.tile_pool(name="rpool", bufs=4) as rpool,
            tc.tile_pool(name="opool", bufs=4) as opool,
            tc.tile_pool(name="qkvps", bufs=2, space="PSUM") as qkvps,
            tc.tile_pool(name="sps", bufs=2, space="PSUM") as sps,
            tc.tile_pool(name="ops", bufs=1, space="PSUM") as ops,
            tc.tile_pool(name="vps", bufs=1, space="PSUM") as vps,
            tc.tile_pool(name="ptps", bufs=2, space="PSUM") as ptps,
        ):
            w_sb = wpool.tile([128, KCH, 3, H], BF16)
            nc.sync.dma_start(
                out=w_sb, in_=w_d.rearrange("(k q) p h -> q k p h", q=128)
            )
            ident = wpool.tile([128, 128], BF16)
            make_identity(nc, ident)
            # Persistent zero-padded Q tiles (double-buffered by hand):
            # q2[:, 0, :] = [QT_b0; 0], q2[:, 1, :] = [0; QT_b1] so scores
            # can contract over the full 128 partitions with base-0 operands.
            q2_tiles = []
            for jq in range(4 if TAIL2 else 2):
                q2 = wpool.tile([128, 2, T], BF16, tag=f"q2_{jq}")
                nc.vector.memset(q2, 0.0)
                q2_tiles.append(q2)

            # Per-pair state for the software pipeline
            st = {}

            def stage_load(g):
                xtg_sb = xtpool.tile([128, GP, 2, KCH, T], BF16)
                nc.gpsimd.dma_start(out=xtg_sb, in_=x_d[g])
                for j in range(GP):
                    st[g * GP + j] = {"xt": xtg_sb[:, j]}

            def stage_proj(i):
                s = st[i]
                xt_sb = s["xt"]
                # col-tiled pair in one PSUM bank: b0 -> partitions 0-63,
                # b1 -> 64-127
                qkv_ps = qkvps.tile([128, 3, T], F32)
                for b in range(2):
                    for p in range(3):
                        for k in range(KCH):
                            nc.tensor.matmul(
                                qkv_ps[64 * b : 64 * b + 64, p, :],
                                w_sb[:, k, p, :],
                                xt_sb[:, b, k, :],
                                start=(k == 0),
                                stop=(k == KCH - 1),
                                tile_position=(0, 64 * b),
                                skip_group_check=True,
                            )
                kv_sb = qkvpool.tile([128, 2, T], BF16)
                nc.scalar.copy(out=kv_sb, in_=qkv_ps[:, 1:3, :])
                q2 = q2_tiles[i % len(q2_tiles)]
                nc.vector.tensor_copy(q2[0:64, 0, :], qkv_ps[0:64, 0, :])
                nc.vector.tensor_copy(q2[64:128, 1, :], qkv_ps[64:128, 0, :])
                s["kv"] = kv_sb
                s["q2"] = q2

            def stage_scores(i):
                s = st[i]
                kv_sb = s["kv"]
                # V natural layout: one full-partition PE transpose of the
                # stacked [VT_b0; VT_b1] block gives [V_b0 | V_b1]
                v_ps = vps.tile([T, 128], BF16)
                nc.tensor.transpose(v_ps, kv_sb[:, 1, :], ident)
                v_sb = vpool.tile([T, 2, 80], BF16)
                nc.vector.tensor_copy(
                    v_sb[:, :, 0:64], v_ps.rearrange("t (b h) -> t b h", b=2)
                )
                nc.gpsimd.memset(v_sb[:, :, 64:65], 1.0)
                s["v"] = v_sb
                # scores: full-K matmuls against zero-padded Q halves
                s_ps = sps.tile([T, 2, T], F32)
                for b in range(2):
                    nc.tensor.matmul(
                        s_ps[:, b, :],
                        s["q2"][:, b, :],
                        kv_sb[:, 0, :],
                        start=True,
                        stop=True,
                    )
                # exp + causal mask (fill 0 where s > t)
                p_sb = ppool.tile([T, 2, T], BF16)
                nc.scalar.activation(
                    out=p_sb, in_=s_ps, func=mybir.ActivationFunctionType.Exp
                )
                nc.gpsimd.affine_select(
                    out=p_sb,
                    in_=p_sb,
                    pattern=[[0, 2], [-1, T]],
                    compare_op=mybir.AluOpType.is_ge,
                    fill=0.0,
                    base=0,
                    channel_multiplier=1,
                )
                s["p"] = p_sb

            def stage_finish(i):
                s = st[i]
                p_sb, v_sb = s["p"], s["v"]
                # P^T on PE
                pt_ps = ptps.tile([T, 2, T], BF16)
                for b in range(2):
                    nc.tensor.transpose(pt_ps[:, b, :], p_sb[:, b, :], ident)
                pt_sb = ptpool.tile([T, 2, T], BF16)
                nc.vector.tensor_copy(pt_sb, pt_ps)
                # out = P @ [V | 1]: col 64 is the softmax denominator
                o_ps = ops.tile([T, 2, H + 1], F32)
                for b in range(2):
                    nc.tensor.matmul(
                        o_ps[:, b, :],
                        pt_sb[:, b, :],
                        v_sb[:, b, 0 : H + 1],
                        start=True,
                        stop=True,
                    )
                # normalize + store
                r_sb = rpool.tile([T, 2, 1], F32)
                nc.vector.reciprocal(out=r_sb, in_=o_ps[:, :, H : H + 1])
                o_sb = opool.tile([T, 2, H], F32)
                nc.vector.tensor_mul(
                    o_sb, o_ps[:, :, 0:H], r_sb.to_broadcast([T, 2, H])
                )
                nc.sync.dma_start(
                    out=out_d[2 * i : 2 * i + 2].rearrange("b t h -> t b h"),
                    in_=o_sb,
                )
                del st[i]

            def stage_scores2(j):
                i0 = 2 * j
                sA, sB = st[i0], st[i0 + 1]
                v_ps = vps.tile([T, 2, 128], BF16)
                for p, s in enumerate((sA, sB)):
                    nc.tensor.transpose(v_ps[:, p, :], s["kv"][:, 1, :], ident)
                v_sb = vpool.tile([T, 4, 80], BF16)
                nc.vector.tensor_copy(
                    v_sb[:, :, 0:64],
                    v_ps.rearrange("t p (b h) -> t (p b) h", b=2),
                )
                nc.gpsimd.memset(v_sb[:, :, 64:65], 1.0)
                sA["v"] = v_sb
                s_ps = sps.tile([T, 4, T], F32)
                for k, s in enumerate((sA, sA, sB, sB)):
                    b = k % 2
                    nc.tensor.matmul(
                        s_ps[:, k, :],
                        s["q2"][:, b, :],
                        s["kv"][:, 0, :],
                        start=True,
                        stop=True,
                    )
                p_sb = ppool.tile([T, 4, T], BF16)
                nc.scalar.activation(
                    out=p_sb, in_=s_ps, func=mybir.ActivationFunctionType.Exp
                )
                nc.gpsimd.affine_select(
                    out=p_sb,
                    in_=p_sb,
                    pattern=[[0, 4], [-1, T]],
                    compare_op=mybir.AluOpType.is_ge,
                    fill=0.0,
                    base=0,
                    channel_multiplier=1,
                )
                sA["p"] = p_sb

            def stage_finish2(j):
                i0 = 2 * j
                sA = st[i0]
                p_sb, v_sb = sA["p"], sA["v"]
                pt_ps = ptps.tile([T, 4, T], BF16)
                for k in range(4):
                    nc.tensor.transpose(pt_ps[:, k, :], p_sb[:, k, :], ident)
                pt_sb = ptpool.tile([T, 4, T], BF16)
                nc.vector.tensor_copy(pt_sb, pt_ps)
                o_ps = ops.tile([T, 4, H + 1], F32)
                for k in range(4):
                    nc.tensor.matmul(
                        o_ps[:, k, :],
                        pt_sb[:, k, :],
                        v_sb[:, k, 0 : H + 1],
                        start=True,
                        stop=True,
                    )
                r_sb = rpool.tile([T, 4, 1], F32)
                nc.vector.reciprocal(out=r_sb, in_=o_ps[:, :, H : H + 1])
                o_sb = opool.tile([T, 4, H], F32)
                nc.vector.tensor_mul(
                    o_sb, o_ps[:, :, 0:H], r_sb.to_broadcast([T, 4, H])
                )
                nc.sync.dma_start(
                    out=out_d[4 * j : 4 * j + 4].rearrange("b t h -> t b h"),
                    in_=o_sb,
                )
                del st[i0], st[i0 + 1]

            # 3-stage software pipeline: proj(i) | scores(i-1) | finish(i-2)
            if TAIL2:
                for i in range(NPAIR + 4):
                    if i < NPAIR:
                        if i % GP == 0:
                            stage_load(i // GP)
                        stage_proj(i)
                    if i >= 2 and i % 2 == 0 and (i - 2) // 2 < NPAIR // 2:
                        stage_scores2((i - 2) // 2)
                    if i >= 4 and i % 2 == 0:
                        stage_finish2((i - 4) // 2)
            else:
                for i in range(NPAIR + 2):
                    if i < NPAIR:
                        if i % GP == 0:
                            stage_load(i // GP)
                        stage_proj(i)
                    if i >= 1 and i - 1 < NPAIR:
                        stage_scores(i - 1)
                    if i >= 2:
                        stage_finish(i - 2)

    nc.compile()
    return nc


def _prep_inputs(x, Wq, Wk, Wv):
    w = np.stack(
        [np.asarray(Wq).T / 8.0, np.asarray(Wk).T, np.asarray(Wv).T], axis=1
    ).astype(ml_dtypes.bfloat16)  # [C, 3, H]
    # [B, T, C] -> per-core, per-group partition-contiguous blocks
    # [core, group, q, pair, b, k, t] where c = k*128 + q
    xt = np.asarray(x, dtype=np.float32).reshape(
        NCORES, NGRP, GP, 2, T, KCH, 128
    )
    xt = np.ascontiguousarray(xt.transpose(0, 1, 6, 2, 3, 5, 4))
    return [{"x": xt[i], "w": w} for i in range(NCORES)]


def _run(in_maps, **kw):
    if "nc" not in _cache:
        _cache["nc"] = _build()
    return run_bass_kernel_spmd(
        _cache["nc"], in_maps, core_ids=list(range(NCORES)), **kw
    )


def kernel(x, Wq, Wk, Wv):
    res = _run(_prep_inputs(x, Wq, Wk, Wv))
    out = np.concatenate([r["out"] for r in res.results], axis=0)
    return np.ascontiguousarray(out.astype(np.float32).reshape(B, T, H))



# revision 2
# speedup vs baseline: 1.2687x; 1.2687x over previous
"""Causal single-head attention, data-parallel across 8 TRN2 NeuronCores.

Problem: x [512, 128, 512] f32, Wq/Wk/Wv [64, 512] f32.
  Q = x @ Wq.T; K = x @ Wk.T; V = x @ Wv.T     (per batch, [T=128, H=64])
  out = softmax(causal(Q K^T / 8)) @ V          ([T, H])

Sharding: batch dim (512) split across 8 cores, 64 batches/core, no
collectives.  Host prep (layout only): x is cast to bf16 and laid out
as [tile, c-part, c-chunk, token] so each 512-token tile (4 batches)
feeds N=512 matmuls directly; weights pre-transposed to [c, 3H] bf16
(unscaled -- the 1/8 softmax scale is folded into the exp activation).

Per-core kernel (16 token-tiles of 4 batches, bf16 compute, f32 out):
  - QK projection: one [128,512]-out matmul per C-chunk (M=128: Q rows
    0-63, K rows 64-127), N=512 amortizes the per-matmul overhead.
  - V projection: col-tiled pairs (tile_position=(0,64g)) so two N=256
    matmuls run concurrently on half-arrays.
  - K must be a base-0 matmul operand (partition-offset operands crash
    this silicon), so a constant shift matrix (ones at (m+64, m)) is
    matmul'd against qk_sb to relocate K rows 64-127 -> 0-63.
  - scores are computed TRANSPOSED: S^T[s,t] = sum_h KT[h,s] QT[h,t]
    via lhsT=K^T, rhs=Q^T (both h-partitioned from the projection), so
    exp(S^T) IS P^T and feeds the PV matmul with no P transpose.
  - V natural layout via one PE transpose per stacked batch-pair.
  - ACT exp applies the 1/8 scale; GPSIMD affine_select applies the
    causal mask (keep t >= s in [s, b, t] layout).
  - V gets a ones column appended so the PV matmul also produces the
    softmax row-sums; DVE reciprocal + broadcast multiply normalizes.
  - 3-stage software pipeline (proj i | mid i-1 | back i-2) keeps the
    PE queue from head-blocking on ACT/DVE work.
"""

import contextlib

import numpy as np
import ml_dtypes

import concourse.mybir as mybir
import concourse.tile as tile
from concourse import bacc
from concourse.bass_utils import run_bass_kernel_spmd
from concourse.masks import make_identity

B, T, C, H = 512, 128, 512, 64
NCORES = 8
BPC = B // NCORES          # 64 batches per core
NBT = 4                    # batches per token tile
NT = BPC // NBT            # 16 token tiles
NTOK = NBT * T             # 512 tokens per tile
KCH = C // 128             # 4 contraction chunks

BF16 = mybir.dt.bfloat16
F32 = mybir.dt.float32

_cache = {}


def _build(reps=1):
    nc = bacc.Bacc(
        "TRN2", target_bir_lowering=False, debug=False, enable_asserts=False
    )
    x_d = nc.dram_tensor(
        "x", [NT, 128, KCH, NTOK], BF16, kind="ExternalInput"
    ).ap()
    w_d = nc.dram_tensor("w", [128, KCH, 3 * H], BF16, kind="ExternalInput").ap()
    out_d = nc.dram_tensor("out", [NT, T, NBT, H], F32, kind="ExternalOutput").ap()

    with tile.TileContext(nc) as tc:
        with (
            tc.tile_pool(name="const", bufs=1) as cpool,
            tc.tile_pool(name="xt", bufs=3) as xtpool,
            tc.tile_pool(name="qk", bufs=2) as qkpool,
            tc.tile_pool(name="ksb", bufs=2) as kpool,
            tc.tile_pool(name="vtsb", bufs=2) as vtpool,
            tc.tile_pool(name="vsb", bufs=2) as vpool,
            tc.tile_pool(name="psb", bufs=2) as ppool,
            tc.tile_pool(name="rsb", bufs=2) as rpool,
            tc.tile_pool(name="osb", bufs=2) as opool,
            tc.tile_pool(name="qkps", bufs=2, space="PSUM") as qkps,
            tc.tile_pool(name="vps", bufs=2, space="PSUM") as vps,
            tc.tile_pool(name="kps", bufs=1, space="PSUM") as kps,
            tc.tile_pool(name="vtps", bufs=1, space="PSUM") as vtps,
            tc.tile_pool(name="sps", bufs=1, space="PSUM") as sps,
            tc.tile_pool(name="ops", bufs=1, space="PSUM") as ops,
        ):
            w_sb = cpool.tile([128, KCH, 3 * H], BF16)
            nc.sync.dma_start(out=w_sb, in_=w_d)
            ident = cpool.tile([128, 128], BF16)
            make_identity(nc, ident)
            # shift64: ones at (m+64, m); matmul(lhsT=shift, rhs=src)
            # relocates src partitions 64-127 to output partitions 0-63.
            shift = cpool.tile([128, 64], BF16)
            nc.gpsimd.memset(shift, 1.0)
            nc.gpsimd.affine_select(
                out=shift,
                in_=shift,
                pattern=[[-1, 64]],
                base=-64,
                channel_multiplier=1,
                compare_op=mybir.AluOpType.is_equal,
                fill=0.0,
            )

            st = {}

            def dma_in(i):
                xt = xtpool.tile([128, KCH, NTOK], BF16)
                nc.gpsimd.dma_start(out=xt, in_=x_d[i])
                st[i] = {"xt": xt}

            def stage_proj(i):
                xt = st[i]["xt"]
                qk_ps = qkps.tile([128, NTOK], F32)
                for j in range(KCH):
                    nc.tensor.matmul(
                        qk_ps,
                        w_sb[:, j, 0:128],
                        xt[:, j, :],
                        start=(j == 0),
                        stop=(j == KCH - 1),
                    )
                v_ps = vps.tile([128, NTOK // 2], F32)
                for j in range(KCH):
                    for g in range(2):
                        nc.tensor.matmul(
                            v_ps[64 * g : 64 * g + 64, :],
                            w_sb[:, j, 128:192],
                            xt[:, j, 256 * g : 256 * g + 256],
                            start=(j == 0),
                            stop=(j == KCH - 1),
                            tile_position=(0, 64 * g),
                            skip_group_check=True,
                        )
                st[i]["qk_ps"] = qk_ps
                st[i]["v_ps"] = v_ps

            def stage_mid(i):
                s = st[i]
                qk_sb = qkpool.tile([128, NTOK], BF16)
                nc.vector.tensor_copy(qk_sb, s["qk_ps"])
                vt_sb = vtpool.tile([128, NTOK // 2], BF16)
                nc.vector.tensor_copy(vt_sb, s["v_ps"])
                # K rows (partitions 64-127) -> base-0 tile via PE shift
                k_ps = kps.tile([64, NTOK], F32)
                nc.tensor.matmul(k_ps, shift, qk_sb, start=True, stop=True)
                k_sb = kpool.tile([64, NTOK], BF16)
                nc.vector.tensor_copy(k_sb, k_ps)
                # V natural layout: transpose stacked pairs [VT_b; VT_b+2]
                vt_ps = vtps.tile([T, 2, 128], BF16)
                for p in range(2):
                    nc.tensor.transpose(
                        vt_ps[:, p, :], vt_sb[:, 128 * p : 128 * p + 128], ident
                    )
                v_sb = vpool.tile([T, NBT, H + 1], BF16)
                nc.vector.tensor_copy(
                    v_sb.rearrange("t (g p) h -> t p g h", g=2)[:, :, :, 0:H],
                    vt_ps.rearrange("t p (g h) -> t p g h", g=2),
                )
                nc.gpsimd.memset(v_sb[:, :, H : H + 1], 1.0)
                # transposed scores: S^T[s,t] per batch, K=64 contraction
                s_ps = sps.tile([T, NBT, T], F32)
                for b in range(NBT):
                    nc.tensor.matmul(
                        s_ps[:, b, :],
                        k_sb[:, T * b : T * b + T],
                        qk_sb[0:64, T * b : T * b + T],
                        start=True,
                        stop=True,
                    )
                p_sb = ppool.tile([T, NBT, T], BF16)
                nc.scalar.activation(
                    out=p_sb,
                    in_=s_ps,
                    func=mybir.ActivationFunctionType.Exp,
                    scale=0.125,
                )
                # causal: keep where t - s >= 0 (layout [s, b, t])
                nc.gpsimd.affine_select(
                    out=p_sb,
                    in_=p_sb,
                    pattern=[[0, NBT], [1, T]],
                    compare_op=mybir.AluOpType.is_ge,
                    fill=0.0,
                    base=0,
                    channel_multiplier=-1,
                )
                s["v"] = v_sb
                s["p"] = p_sb

            def stage_back(i):
                s = st[i]
                o_ps = ops.tile([T, NBT, H + 1], F32)
                for b in range(NBT):
                    nc.tensor.matmul(
                        o_ps[:, b, :],
                        s["p"][:, b, :],
                        s["v"][:, b, :],
                        start=True,
                        stop=True,
                    )
                r_sb = rpool.tile([T, NBT, 1], F32)
                nc.vector.reciprocal(out=r_sb, in_=o_ps[:, :, H : H + 1])
                o_sb = opool.tile([T, NBT, H], F32)
                nc.vector.tensor_mul(
                    o_sb, o_ps[:, :, 0:H], r_sb.to_broadcast([T, NBT, H])
                )
                nc.sync.dma_start(out=out_d[i], in_=o_sb)
                del st[i]

            loop = (
                tc.For_i(0, reps, 1, hint_engines=tuple(nc.engines))
                if reps > 1
                else contextlib.nullcontext()
            )
            with loop:
                for i in range(NT + 2):
                    if i < NT:
                        if i == 0:
                            dma_in(0)
                            dma_in(1)
                            dma_in(2)
                        elif i + 2 < NT:
                            dma_in(i + 2)
                        stage_proj(i)
                    if 1 <= i <= NT:
                        stage_mid(i - 1)
                    if i >= 2:
                        stage_back(i - 2)

    nc.compile()
    return nc


def _prep_inputs(x, Wq, Wk, Wv):
    w = np.concatenate(
        [np.asarray(Wq).T, np.asarray(Wk).T, np.asarray(Wv).T], axis=1
    )  # [C, 3H]
    w = np.ascontiguousarray(
        w.reshape(KCH, 128, 3 * H).transpose(1, 0, 2)
    ).astype(ml_dtypes.bfloat16)  # [128, KCH, 3H]
    # x [B, T, C] -> per-core [NT, 128(c-part), KCH, NTOK], token = b*T + t
    xt = np.asarray(x, dtype=np.float32).reshape(NCORES, NT, NBT, T, KCH, 128)
    xt = np.ascontiguousarray(xt.transpose(0, 1, 5, 4, 2, 3)).astype(
        ml_dtypes.bfloat16
    )
    xt = xt.reshape(NCORES, NT, 128, KCH, NTOK)
    return [{"x": xt[i], "w": w} for i in range(NCORES)]


def _run(in_maps, **kw):
    if "nc" not in _cache:
        _cache["nc"] = _build()
    return run_bass_kernel_spmd(
        _cache["nc"], in_maps, core_ids=list(range(NCORES)), **kw
    )


def kernel(x, Wq, Wk, Wv):
    res = _run(_prep_inputs(x, Wq, Wk, Wv))
    outs = []
    for r in res.results:
        o = r["out"].astype(np.float32)  # [NT, T, NBT, H]
        outs.append(o.transpose(0, 2, 1, 3).reshape(BPC, T, H))
    return np.ascontiguousarray(np.concatenate(outs, axis=0))
